# revision 6
# baseline (speedup 1.0000x reference)
"""Multi-head self-attention (GroupNorm -> qkv -> attention -> proj) on 8 trn2 cores.

Sharding: each core owns (batch b = core//4, query-chunk q = core%4 of 1024
pixels). GroupNorm stats and K/V are computed redundantly per core (cheap);
queries and the attention matrix are sharded by (batch, query-range), so no
collectives are needed. Per-core inputs are column-rolled so every core's
query range is columns [0:1024] of its own x (softmax over keys is
permutation-invariant, so rolling pixels does not change results).

Device pipeline per core (all 16-bit tensors fp16; matmul accum fp32):
  x arrives in 8 column-chunk DMAs on the sync queue while packed weights/
  consts stream on the scalar queue; per chunk: fp16 cast (split ACT/DVE)
  + bn_stats. A 16-matmul contiguous warm-up burst lifts the PE HAM clock
  gate to 8/8 (2.4 GHz), and dependency-free filler matmuls (into a
  dedicated scratch PSUM bank pair) are threaded through GroupNorm and the
  whole attention phase so the gate never drops back to 4/8 — a cold PE
  halves matmul rate and, once cold at ~80% busy, never re-warms.
  GroupNorm is an affine s*x+t per channel, folded into the qkv weights on
  device (W' = W·diag(s), bias' = b + W·t); hn is never materialized. The
  v-half of the fold (Wv·t) is added post-softmax (attention weights sum
  to 1); k bias is dropped (per-query score constants cancel in softmax).
  Attention, flash-style without max-subtraction (scores in [-8, 8], exp
  safe): per (head-pair, q-chunk 512, k-chunk 128): S^T row-tiled into a
  2-bank PSUM tile; exp split between ACT (LUT Exp, ~73%) and DVE (custom
  cubic^64 two-pass, ~27%) so both engines saturate; AV + fused Z
  (ones-columns of vT broadcast the softmax denominator) accumulate into a
  persistent 2-bank oz tile; normalize via reciprocal_approx_fast.
  o = o'/Z + (bv + Wv t), then out = Wp o + bp -> [256, 1024] fp32.
"""

import ml_dtypes
import numpy as np

F16 = np.float16

C = 256
N = 4096
NQ = 1024
NH = 8
HD = 32
G = 8
EPS = 1e-5
P = 128
QC = 512
NKC = N // P  # 32 k-chunks
SCALE = HD ** -0.5
NCORES = 8

# fraction of exp tiles routed to the DVE two-pass path (Bresenham over the
# 256-tile stream); the rest go to ACT's LUT Exp.
DVE_NUM = 7
DVE_DEN = 26
# dependency-free PE filler matmuls per k-chunk step (keeps HAM at 8/8)
FILL_KC = 4

_CACHE = {}


def _build_program():
    import concourse.bass as bass  # noqa: F401
    import concourse.tile as tile
    from concourse import bacc, mybir
    from concourse import dve_ops as dv
    from concourse.dve_spec import C0, C1, C2, One, Spec, Src0, sq
    from concourse.dve_ops import DveOp

    def reg_dve(name, spec):
        for op in dv.OPS:
            if op.name == name:
                return op
        op = DveOp(name, spec, subdim=False, uops_sha={})
        dv.OPS.append(op)
        dv.CUSTOM_DVE_SPECS[name] = spec
        dv._SUB_OPCODE_FOR_NAME[name] = dv._CUSTOM_DVE_ROW_BASE + len(dv.OPS) - 1
        for ver in ("v3", "v4"):
            try:
                op.compile(ver)
            except ValueError as e:
                op.uops_sha[ver] = str(e).split(f"{ver}: ")[1].split(" ")[0]
                op.compile(ver)
        return op

    # exp(x*SCALE) = (q(x))^64 with q a cubic fit of exp(x*SCALE/64),
    # constant term pinned to 1. op1 computes q(x)^4 (8 ALU stages), op2
    # raises to the 16th power. Validated on HW: max rel err 1.1e-4 in f32.
    E4 = reg_dve("ANT_EXP64_P4",
                 Spec(body=sq(sq(((C0 * Src0 + C1) * Src0 + C2) * Src0 + One))))
    S16 = reg_dve("ANT_SQ4", Spec(body=sq(sq(sq(sq(Src0))))))
    EC3 = 3.4064999134215023e-09
    EC2 = 3.8173198326396545e-06
    EC1 = 0.0027622298856143134

    F32 = mybir.dt.float32
    FP16 = mybir.dt.float16
    AF = mybir.ActivationFunctionType
    ALU = mybir.AluOpType

    nc = bacc.Bacc("TRN2", target_bir_lowering=False)

    x_d = nc.declare_dram_parameter("x", [C, N], F32, isOutput=False)
    # packed weights: [wqT | wkT | wvT | wpT] along columns
    wall_d = nc.declare_dram_parameter("wall", [C, 4 * C], FP16, isOutput=False)
    # packed per-channel consts: [bq | bv | bp | gnw | gnb]
    cst_d = nc.declare_dram_parameter("cst", [C, 5], F32, isOutput=False)
    gsel_d = nc.declare_dram_parameter("gsel", [P, 4], FP16, isOutput=False)
    gselT_d = nc.declare_dram_parameter("gselT", [4, P], FP16, isOutput=False)
    out_d = nc.declare_dram_parameter("out", [C, NQ], F32, isOutput=True)

    XCH = 4          # x column-chunks per 128-row half
    XW = N // XCH    # 1024 columns per chunk

    with tile.TileContext(nc) as tc:
        const = tc.alloc_tile_pool(name="const", bufs=1)
        big = tc.alloc_tile_pool(name="big", bufs=1)
        work = tc.alloc_tile_pool(name="work", bufs=2)
        expp = tc.alloc_tile_pool(name="expp", bufs=4)
        etpp = tc.alloc_tile_pool(name="etpp", bufs=2)
        psp = tc.alloc_tile_pool(name="psp", bufs=2, space="PSUM")
        filp = tc.alloc_tile_pool(name="filp", bufs=1, space="PSUM")
        ozp = tc.alloc_tile_pool(name="ozp", bufs=1, space="PSUM")

        # ---- x load (chunked, sync queue) + weights/consts (scalar queue) ----
        x_sb = [[big.tile([P, XW], F32, name=f"x{t}_{c}", tag=f"x{t}_{c}")
                 for c in range(XCH)] for t in range(2)]
        x16 = [big.tile([P, N], FP16, name=f"x16_{t}", tag=f"x16_{t}")
               for t in range(2)]
        for c in range(XCH):
            for t in range(2):
                nc.sync.dma_start(out=x_sb[t][c],
                                  in_=x_d[t * P:(t + 1) * P, c * XW:(c + 1) * XW])

        wall_sb = [const.tile([P, 4 * C], FP16, name=f"wall{i}", tag=f"wall{i}")
                   for i in range(2)]
        cst_sb = [const.tile([P, 5], F32, name=f"cst{i}", tag=f"cst{i}")
                  for i in range(2)]
        gsel_sb = const.tile([P, 4], FP16, name="gsel", tag="gsel")
        gselT_sb = const.tile([4, P], FP16, name="gselT", tag="gselT")
        for i in range(2):
            sl = slice(i * P, (i + 1) * P)
            nc.scalar.dma_start(out=wall_sb[i], in_=wall_d[sl, :])
            nc.scalar.dma_start(out=cst_sb[i], in_=cst_d[sl, :])
        nc.scalar.dma_start(out=gsel_sb, in_=gsel_d[:, :])
        nc.scalar.dma_start(out=gselT_sb, in_=gselT_d[:, :])
        wq_sb = [wall_sb[i][:, 0:C] for i in range(2)]
        wk_sb = [wall_sb[i][:, C:2 * C] for i in range(2)]
        wv_sb = [wall_sb[i][:, 2 * C:3 * C] for i in range(2)]
        wp_sb = [wall_sb[i][:, 3 * C:4 * C] for i in range(2)]
        bq_sb = [cst_sb[i][:, 0:1] for i in range(2)]
        bv_sb = [cst_sb[i][:, 1:2] for i in range(2)]
        bp_sb = [cst_sb[i][:, 2:3] for i in range(2)]
        gnw_sb = [cst_sb[i][:, 3:4] for i in range(2)]
        gnb_sb = [cst_sb[i][:, 4:5] for i in range(2)]
        eps_sb = const.tile([4, 1], F32, name="eps", tag="eps")
        nc.vector.memset(eps_sb, EPS)

        # vT3[p, kc, head, 0:32] = v^T channels; cols 32:64 = 1.0 (fused Z
        # accumulation: the M=64 AV matmul emits o' rows 0-31 and Z broadcast
        # to rows 32-63).
        vT_sb = big.tile([P, NKC, NH, 64], FP16, name="vt", tag="vt")
        nc.gpsimd.memset(vT_sb[:, :, :, 32:64], 1.0)

        fil = filp.tile([P, 2 * QC], F32, name="fil", tag="fil")

        def filler(n):
            for _ in range(n):
                nc.tensor.matmul(out=fil[:, 0:QC],
                                 lhsT=x16[0][:, 0:P],
                                 rhs=x16[0][:, 0:QC],
                                 start=True, stop=True)

        # ---- fp16 cast (ACT for t=1, DVE for t=0) + GroupNorm stats; PE
        # warm-up burst after the first chunk, trickle through the rest ----
        stats = [work.tile([P, 2 * XCH, 6], F32, name=f"gnstats{t}", tag=f"gnstats{t}")
                 for t in range(2)]
        for c in range(XCH):
            nc.vector.tensor_copy(out=x16[0][:, c * XW:(c + 1) * XW],
                                  in_=x_sb[0][c])
            nc.scalar.copy(out=x16[1][:, c * XW:(c + 1) * XW], in_=x_sb[1][c])
            for t in range(2):
                xv = x_sb[t][c].rearrange("p (a b) -> p a b", b=512)
                for j in range(2):
                    nc.vector.bn_stats(out=stats[t][:, 2 * c + j, :],
                                       in_=xv[:, j, :])
            filler(16 if c == 0 else 6)

        # ---- GroupNorm aggregation -> per-channel affine s,t -> fold into
        # weights: wX2 = wX * s (rows = in-channels), biases via W @ t ----
        s_t = [work.tile([P, 1], F32, name=f"gns{t}", tag=f"gns{t}") for t in range(2)]
        t16 = [work.tile([P, 1], FP16, name=f"gnt16{t}", tag=f"gnt16{t}") for t in range(2)]
        for t in range(2):
            mv = work.tile([P, 2], F32, name="gnmv", tag="gnmv")
            nc.vector.bn_aggr(out=mv, in_=stats[t])
            # st2 = (mean_c, E[x^2]_c) fp16 for the selector matmul
            st2 = work.tile([P, 2], FP16, name="gnst2", tag="gnst2")
            e2f = work.tile([P, 1], F32, name="gne2", tag="gne2")
            nc.vector.tensor_mul(out=e2f, in0=mv[:, 0:1], in1=mv[:, 0:1])
            nc.vector.tensor_add(out=e2f, in0=e2f, in1=mv[:, 1:2])
            nc.vector.tensor_copy(out=st2[:, 0:1], in_=mv[:, 0:1])
            nc.vector.tensor_copy(out=st2[:, 1:2], in_=e2f)
            # group combine: [4,2] = (mean_g, E2_g) via selector matmul (1/32)
            gp = psp.tile([P, 2 * QC], F32, name="psgn", tag="ps")
            nc.tensor.matmul(out=gp[0:4, 0:2], lhsT=gsel_sb, rhs=st2,
                             start=True, stop=True)
            filler(2)
            vg = work.tile([4, 1], F32, name="gnvg", tag="gnvg")
            gm = work.tile([4, 2], F32, name="gngm", tag="gngm")
            g2 = work.tile([4, 2], FP16, name="gng2", tag="gng2")
            nc.vector.tensor_copy(out=gm, in_=gp[0:4, 0:2])
            nc.vector.tensor_copy(out=g2[:, 0:1], in_=gm[:, 0:1])
            nc.vector.tensor_mul(out=vg, in0=gm[:, 0:1], in1=gm[:, 0:1])
            nc.vector.tensor_sub(out=vg, in0=gm[:, 1:2], in1=vg)
            nc.scalar.activation(out=vg, in_=vg, func=AF.Sqrt, bias=eps_sb)
            nc.vector.reciprocal(out=vg, in_=vg)
            nc.vector.tensor_copy(out=g2[:, 1:2], in_=vg)
            # broadcast to channels: [128,2] = (mean_c', rstd_c')
            bc = psp.tile([P, 2 * QC], F32, name="psgn2", tag="ps")
            nc.tensor.matmul(out=bc[:, 0:2], lhsT=gselT_sb, rhs=g2,
                             start=True, stop=True)
            filler(2)
            t_t = work.tile([P, 1], F32, name="gnt", tag="gnt")
            nc.vector.tensor_mul(out=s_t[t], in0=bc[:, 1:2], in1=gnw_sb[t])
            nc.vector.tensor_mul(out=t_t, in0=bc[:, 0:1], in1=s_t[t])
            nc.vector.tensor_sub(out=t_t, in0=gnb_sb[t], in1=t_t)
            nc.vector.tensor_copy(out=t16[t], in_=t_t)

        # scaled weights (GN fold): rows are input channels
        wq2 = [const.tile([P, C], FP16, name=f"wq2_{i}", tag=f"wq2_{i}") for i in range(2)]
        wk2 = [const.tile([P, C], FP16, name=f"wk2_{i}", tag=f"wk2_{i}") for i in range(2)]
        wv2 = [const.tile([P, C], FP16, name=f"wv2_{i}", tag=f"wv2_{i}") for i in range(2)]
        for i in range(2):
            nc.vector.tensor_scalar(out=wq2[i], in0=wq_sb[i], scalar1=s_t[i],
                                    scalar2=None, op0=ALU.mult)
            nc.vector.tensor_scalar(out=wk2[i], in0=wk_sb[i], scalar1=s_t[i],
                                    scalar2=None, op0=ALU.mult)
            nc.vector.tensor_scalar(out=wv2[i], in0=wv_sb[i], scalar1=s_t[i],
                                    scalar2=None, op0=ALU.mult)
            filler(2)
        # folded biases: bq' = bq + Wq@t  (into q);  bvv = bv + Wv@t (post-
        # softmax, since attention weights sum to 1). k bias dropped.
        psb = psp.tile([P, 2 * QC], F32, name="psb", tag="ps")
        for rc in range(2):
            for cc in range(2):
                nc.tensor.matmul(out=psb[:, 2 * rc:2 * rc + 1],
                                 lhsT=wq_sb[cc][:, rc * P:(rc + 1) * P],
                                 rhs=t16[cc], start=(cc == 0), stop=(cc == 1))
            for cc in range(2):
                nc.tensor.matmul(out=psb[:, 2 * rc + 1:2 * rc + 2],
                                 lhsT=wv_sb[cc][:, rc * P:(rc + 1) * P],
                                 rhs=t16[cc], start=(cc == 0), stop=(cc == 1))
        filler(2)
        bq2 = [work.tile([P, 1], F32, name=f"bq2_{i}", tag=f"bq2_{i}") for i in range(2)]
        bvv = [work.tile([P, 1], F32, name=f"bvv{i}", tag=f"bvv{i}") for i in range(2)]
        for rc in range(2):
            nc.vector.tensor_add(out=bq2[rc], in0=psb[:, 2 * rc:2 * rc + 1],
                                 in1=bq_sb[rc])
            nc.vector.tensor_add(out=bvv[rc], in0=psb[:, 2 * rc + 1:2 * rc + 2],
                                 in1=bv_sb[rc])

        # ---- qkv + attention (emission interleaved: q/k for head-group 0
        # first, vT blocks streamed inside the first attention pass, q/k for
        # head-group 1 after head-group 0) ----
        q_sb = [big.tile([P, NQ], FP16, name=f"q{i}", tag=f"q{i}") for i in range(2)]
        k_sb = [big.tile([P, N], FP16, name=f"k{i}", tag=f"k{i}") for i in range(2)]

        def emit_q(hg, on_act):
            for qcc in range(NQ // QC):
                ps = psp.tile([P, 2 * QC], F32, name="psq", tag="ps")
                for cc in range(2):
                    nc.tensor.matmul(
                        out=ps[:, 0:QC],
                        lhsT=wq2[cc][:, hg * P:(hg + 1) * P],
                        rhs=x16[cc][:, qcc * QC:(qcc + 1) * QC],
                        start=(cc == 0), stop=(cc == 1))
                dst = q_sb[hg][:, qcc * QC:(qcc + 1) * QC]
                if on_act:
                    nc.scalar.activation(out=dst, in_=ps[:, 0:QC],
                                         func=AF.Identity, bias=bq2[hg])
                else:
                    nc.vector.tensor_scalar(out=dst, in0=ps[:, 0:QC],
                                            scalar1=bq2[hg], scalar2=None,
                                            op0=ALU.add)

        def emit_k(hg, act_mask):
            for ncc in range(N // (2 * QC)):
                ps = psp.tile([P, 2 * QC], F32, name="psk", tag="ps")
                for half in range(2):
                    for cc in range(2):
                        nc.tensor.matmul(
                            out=ps[:, half * QC:(half + 1) * QC],
                            lhsT=wk2[cc][:, hg * P:(hg + 1) * P],
                            rhs=x16[cc][:, (2 * ncc + half) * QC:
                                        (2 * ncc + half + 1) * QC],
                            start=(cc == 0), stop=(cc == 1))
                dst = k_sb[hg][:, 2 * ncc * QC:2 * (ncc + 1) * QC]
                if (act_mask >> ncc) & 1:
                    nc.scalar.copy(out=dst, in_=ps)
                else:
                    nc.vector.tensor_copy(out=dst, in_=ps)

        def emit_vt(ncc2):
            # two 128-pixel chunks -> one [128, 512] PSUM region -> one cast
            ps = psp.tile([P, 2 * QC], F32, name="psv", tag="ps")
            for j in range(2):
                for cc in range(2):
                    nc.tensor.matmul(
                        out=ps[:, j * C:(j + 1) * C],
                        lhsT=x16[cc][:, (2 * ncc2 + j) * P:(2 * ncc2 + j + 1) * P],
                        rhs=wv2[cc],
                        start=(cc == 0), stop=(cc == 1))
            nc.vector.tensor_copy(
                out=vT_sb[:, 2 * ncc2:2 * ncc2 + 2, :, 0:32],
                in_=ps[:, 0:2 * C].rearrange("p (j h d) -> p j h d", j=2, h=NH))

        emit_q(0, on_act=True)
        emit_k(0, act_mask=0b0101)

        # ---- attention ----
        # Granule = (head pair, k-chunk): S^T [128k, 2x512q] -> one 2-bank
        # PSUM tile (bufs=2); persistent 2-bank oz accumulates o' and (via
        # the fused ones-columns) Z across all 32 k-chunks. Exp alternates
        # between ACT (LUT Exp) and DVE (custom cubic^64 two-pass) per
        # DVE_NUM/DVE_DEN so both engines stay saturated. AV emission lags 2
        # granules so the PE never waits on exp; FILL_KC dependency-free
        # fillers per k-chunk keep the PE clock gate open.
        o_flat = [big.tile([P, NQ], FP16, name=f"of{i}", tag=f"of{i}")
                  for i in range(2)]
        out_sb = [big.tile([P, NQ], F32, name=f"out{i}", tag=f"out{i}")
                  for i in range(2)]
        dve_acc = 0
        for hg in range(2):
            for qc in range(NQ // QC):
                qoff = qc * QC
                oz = ozp.tile([P, 2 * QC], F32, name="oz", tag="oz")
                pending = []

                def do_av(item, oz=oz, hg=hg):
                    # h = 2*pr + h' -> oz quadrant: rows 64*(h%2), bank h//2.
                    est_, kc_, pr_ = item
                    for h2 in range(2):
                        h = 2 * pr_ + h2
                        hh = hg * 4 + h
                        nc.tensor.matmul(
                            out=oz[64 * (h % 2):64 * (h % 2) + 64,
                                   QC * (h // 2):QC * (h // 2) + QC],
                            lhsT=vT_sb[:, kc_, hh, :],
                            rhs=est_[:, h2 * QC:(h2 + 1) * QC],
                            start=(kc_ == 0), stop=(kc_ == NKC - 1),
                            tile_position=(0, 64 * (h % 2)))

                for kc in range(NKC):
                    filler(8 if kc == 0 else FILL_KC)
                    if hg == 0 and qc == 0 and kc % 2 == 0:
                        emit_vt(kc // 2)
                    for pr in range(2):
                        S = psp.tile([P, 2 * QC], F32, name="ps", tag="ps")
                        for h in range(2):
                            row = 64 * pr + 32 * h
                            nc.tensor.matmul(
                                out=S[:, h * QC:(h + 1) * QC],
                                lhsT=k_sb[hg][row:row + 32, kc * P:(kc + 1) * P],
                                rhs=q_sb[hg][row:row + 32, qoff:qoff + QC],
                                start=True, stop=True, tile_position=(row, 0))
                        est = expp.tile([P, 2 * QC], FP16, name="est",
                                        tag="est", bufs=4)
                        dve_acc += DVE_NUM
                        if dve_acc >= DVE_DEN:
                            dve_acc -= DVE_DEN
                            etmp = etpp.tile([P, 2 * QC], F32, name="etmp",
                                             tag="etmp", bufs=2)
                            nc.vector._custom_dve(E4, out=etmp, in0=S[:],
                                                  s0=EC3, s1=EC2, imm2=EC1)
                            nc.vector._custom_dve(S16, out=est, in0=etmp)
                        else:
                            nc.scalar.activation(out=est, in_=S[:],
                                                 func=AF.Exp, scale=SCALE)
                        pending.append((est, kc, pr))
                        if len(pending) > 2:
                            do_av(pending.pop(0))
                for item in pending:
                    do_av(item)
                # normalize: unpack oz quadrants via DMA, then
                # o = o'/Z + (bv + Wv t)
                ozc = work.tile([P, 2 * QC], F32, name="ozc", tag="ozc")
                nc.vector.tensor_copy(out=ozc, in_=oz)
                oP = work.tile([P, QC], F32, name="oP", tag="oP")
                zB = work.tile([P, QC], F32, name="zB", tag="zB")
                for h in range(4):
                    r0 = 64 * (h % 2)
                    c0 = QC * (h // 2)
                    nc.sync.dma_start(
                        out=oP[32 * h:32 * h + 32, :],
                        in_=ozc[r0:r0 + 32, c0:c0 + QC])
                    nc.sync.dma_start(
                        out=zB[32 * h:32 * h + 32, :],
                        in_=ozc[r0 + 32:r0 + 64, c0:c0 + QC])
                zr = work.tile([P, QC], F32, name="zr", tag="zr")
                nc.vector.reciprocal_approx_fast(out=zr, in_=zB)
                osl = o_flat[hg][:, qoff:qoff + QC]
                nc.vector.tensor_mul(out=osl, in0=oP, in1=zr)
                nc.vector.tensor_scalar(out=osl, in0=osl, scalar1=bvv[hg],
                                        scalar2=None, op0=ALU.add)
            if hg == 0:
                emit_q(1, on_act=False)
                emit_k(1, act_mask=0b0000)

        # ---- proj ----
        for rc in range(2):
            for ncc in range(NQ // QC):
                ps = psp.tile([P, 2 * QC], F32, name="psp2", tag="ps")
                for cc in range(2):
                    nc.tensor.matmul(
                        out=ps[:, 0:QC],
                        lhsT=wp_sb[cc][:, rc * P:(rc + 1) * P],
                        rhs=o_flat[cc][:, ncc * QC:(ncc + 1) * QC],
                        start=(cc == 0), stop=(cc == 1))
                nc.scalar.activation(
                    out=out_sb[rc][:, ncc * QC:(ncc + 1) * QC], in_=ps[:, 0:QC],
                    func=AF.Identity, bias=bp_sb[rc])
            nc.sync.dma_start(out=out_d[rc * P:(rc + 1) * P, :], in_=out_sb[rc])

        for pool in (ozp, filp, psp, etpp, expp, work, big, const):
            pool.release()

    nc.compile()
    return nc


def kernel(x, gn_weight, gn_bias, w_qkv, b_qkv, w_proj, b_proj):
    from concourse.bass_utils import run_bass_kernel_spmd

    x = np.asarray(x, dtype=np.float32)
    B = x.shape[0]
    xf = x.reshape(B, C, N)

    w_qkv = np.asarray(w_qkv, np.float32)
    wall = np.concatenate([
        w_qkv[0:C, :].T, w_qkv[C:2 * C, :].T, w_qkv[2 * C:3 * C, :].T,
        np.asarray(w_proj, np.float32).T], axis=1).astype(F16)
    wall = np.ascontiguousarray(wall)
    b_qkv = np.asarray(b_qkv, np.float32)
    cst = np.stack([
        b_qkv[0:C], b_qkv[2 * C:3 * C], np.asarray(b_proj, np.float32),
        np.asarray(gn_weight, np.float32), np.asarray(gn_bias, np.float32)],
        axis=1)
    cst = np.ascontiguousarray(cst.astype(np.float32))
    gsel = np.zeros((P, 4), F16)
    for c in range(P):
        gsel[c, c // HD] = 1.0 / HD
    gselT = np.zeros((4, P), F16)
    for c in range(P):
        gselT[c // HD, c] = 1.0

    shared = dict(wall=wall, cst=cst, gsel=gsel, gselT=gselT)
    in_maps = []
    for core in range(NCORES):
        b = core // 4
        roff = (core % 4) * NQ
        xr = np.roll(xf[b], -roff, axis=1)
        m = dict(shared)
        m["x"] = np.ascontiguousarray(xr)
        in_maps.append(m)

    if "nc" not in _CACHE:
        _CACHE["nc"] = _build_program()
    nc = _CACHE["nc"]

    res = run_bass_kernel_spmd(nc, in_maps, list(range(NCORES)))
    _CACHE["last_result"] = res
    out = np.empty((B, C, N), np.float32)
    for core in range(NCORES):
        b = core // 4
        roff = (core % 4) * NQ
        out[b][:, roff:roff + NQ] = np.asarray(res.results[core]["out"])
    return out.reshape(B, C, 64, 64)
